# revision 1
# baseline (speedup 1.0000x reference)
"""Blockwise 16x16 2D DCT on TRN2, 8-core data-parallel.

For each 16x16 tile T of x (32,3,1024,1024): out = K @ T @ K^T.

Per-core Bass pipeline (core shard = 4 images -> (12288, 1024) fp32):
  per 128-row strip (128, 1024):
    mm1   : Y = BD @ X          (BD = blockdiag(K x8), K=128 contraction,
                                 f32r operands = 1 cyc/row on the PE)
    evac  : ACT copies Y PSUM->SBUF (frees PSUM early, feeds DVE from SBUF)
    tr1   : Yt = innerT32(Y)    (DVE 32x32 stream transpose, SBUF->SBUF 2x)
    mm2   : Zt[32a:32a+32] = BD32^T.T @ Yt[32a:32a+32]  (4 packed K=32
            fp32 matmuls at tile_position (32a,32a), concurrent in the array)
    tr2   : Z = innerT32(Zt)    (DVE, fused with PSUM evacuation)
  innerT32 composed with the 32-blockdiag matmuls is algebraically the full
  blockwise DCT (verified exactly in numpy and CoreSim).  Loads issue on the
  SP HWDGE ring and stores on the scalar HWDGE ring so a store waiting on
  compute never delays the next strip's prefetch.  Measured ~211us per core
  (~100MB/core of HBM traffic, beyond the nominal per-core HBM spec); rel err 1.4e-4.
"""

import numpy as np

import concourse.bass as bass
import concourse.bacc as bacc
import concourse.mybir as mybir
from concourse.tile import TileContext
from concourse.bass_utils import run_bass_kernel_spmd

# Problem constants (hardcoded per harness contract)
B, C, H, W = 32, 3, 1024, 1024
KSIZE = 16
NCORES = 8
ROWS = (B // NCORES) * C * H  # 12288 rows per core
F32 = mybir.dt.float32
F32R = mybir.dt.float32r

# matmul operand mode: float32r streams 1 col/cycle on the PE (vs 4 for fp32)
MM_DTYPE = F32R
# stage-2 in f32r is blocked by HW: StreamTranspose rejects f32r operands
# (s4d4_tr_same_src_dst_type), so stage 2 runs fp32 (4 cyc/row, hidden by
# the 4-way tile_position concurrency)
STAGE2_F32R = False


def build_nc(rows=ROWS, width=W, mm_dtype=MM_DTYPE, repeat=1, spt=1,
             s2_f32r=STAGE2_F32R):
    """spt = 128-row strips per SBUF tile (tile free dim = spt*width)."""
    assert rows % (128 * spt) == 0 and width % 1024 == 0
    n_strips = rows // (128 * spt)
    twidth = spt * width
    s2_dtype = mm_dtype if s2_f32r else F32
    nc = bacc.Bacc("TRN2", target_bir_lowering=False, debug=False)
    x = nc.declare_dram_parameter("x", [rows, width], mm_dtype, isOutput=False)
    bdT = nc.declare_dram_parameter("bdT", [128, 128], mm_dtype, isOutput=False)
    bd32T = nc.declare_dram_parameter("bd32T", [128, 32], s2_dtype, isOutput=False)
    out = nc.declare_dram_parameter("out", [rows, width], F32, isOutput=True)

    with TileContext(nc) as tc:
        with (
            tc.tile_pool(name="const", bufs=1) as const_pool,
            tc.tile_pool(name="xin", bufs=6) as xin_pool,
            tc.tile_pool(name="yf", bufs=4) as yf_pool,
            tc.tile_pool(name="yt", bufs=4) as yt_pool,
            tc.tile_pool(name="zout", bufs=4) as zout_pool,
            tc.tile_pool(name="py", bufs=2, space="PSUM") as py_pool,
            tc.tile_pool(name="pz", bufs=2, space="PSUM") as pz_pool,
        ):
            bdT_s = const_pool.tile([128, 128], mm_dtype)
            nc.sync.dma_start(out=bdT_s[:], in_=bdT[:])
            bd32T_s = const_pool.tile([128, 32], s2_dtype)
            nc.sync.dma_start(out=bd32T_s[:], in_=bd32T[:])

            xr = x[:].rearrange("(s q p) w -> s p q w", q=spt, p=128)
            outr = out[:].rearrange("(s q p) w -> s p q w", q=spt, p=128)

            def split_q(ap):
                return ap.rearrange("p (q w) -> p q w", q=spt)

            def strip_body(s):
                # loads on the SP HWDGE ring; stores on the scalar HWDGE ring
                # so a store waiting on compute never blocks the next prefetch
                # (gpsimd SWDGE deadlocks against the busy DVE shared port)
                x_tile = xin_pool.tile([128, twidth], mm_dtype)
                nc.sync.dma_start(out=split_q(x_tile[:]), in_=xr[s])
                z_tile = zout_pool.tile([128, twidth], F32)
                # process in (128, 1024) groups = 2 PSUM banks at a time
                for g in range(twidth // 1024):
                    gsl = slice(g * 1024, (g + 1) * 1024)
                    psum_y = py_pool.tile([128, 1024], F32)
                    psum_z = pz_pool.tile([128, 1024], F32)
                    yf_tile = yf_pool.tile([128, 1024], s2_dtype)
                    yt_tile = yt_pool.tile([128, 1024], s2_dtype)
                    for h in range(2):  # 512-wide chunks (one PSUM bank each)
                        ps = h * 512
                        nc.tensor.matmul(
                            out=psum_y[:, ps:ps + 512],
                            lhsT=bdT_s[:],
                            rhs=x_tile[:, g * 1024 + ps:g * 1024 + ps + 512],
                            start=True, stop=True,
                        )
                    # ACT evacuates Y so the DVE transpose reads SBUF (2x
                    # mode) instead of PSUM (1x)
                    nc.scalar.copy(out=yf_tile[:], in_=psum_y[:])
                    nc.vector.transpose(out=yt_tile[:], in_=yf_tile[:])
                    for h in range(2):
                        ps = h * 512
                        for a in range(4):
                            pa = 32 * a
                            nc.tensor.matmul(
                                out=psum_z[pa:pa + 32, ps:ps + 512],
                                lhsT=bd32T_s[pa:pa + 32, :],
                                rhs=yt_tile[pa:pa + 32, ps:ps + 512],
                                start=True, stop=True,
                                tile_position=(pa, pa),
                            )
                    # final inner-transpose fused with PSUM evacuation (DVE 1x)
                    nc.vector.transpose(out=z_tile[:, gsl], in_=psum_z[:])
                nc.scalar.dma_start(out=outr[s], in_=split_q(z_tile[:]))

            if repeat == 1:
                for s in range(n_strips):
                    strip_body(s)
            else:
                with tc.For_i(0, repeat, 1):
                    for s in range(n_strips):
                        strip_body(s)
    nc.compile()
    return nc


def make_mats(k: np.ndarray):
    k = np.asarray(k, dtype=np.float32)
    ks = k.shape[0]
    bd = np.zeros((128, 128), np.float32)
    for b in range(128 // ks):
        bd[b * ks:(b + 1) * ks, b * ks:(b + 1) * ks] = k
    bd32 = np.zeros((32, 32), np.float32)
    for b in range(32 // ks):
        bd32[b * ks:(b + 1) * ks, b * ks:(b + 1) * ks] = k
    bdT = np.ascontiguousarray(bd.T)
    bd32T = np.ascontiguousarray(np.concatenate([bd32.T] * 4, axis=0))
    return bdT, bd32T


TRACE = False  # test harness hook: set True to profile (NTFF -> perfetto)
LAST_RESULTS = None  # BassKernelResults of the last kernel() call


def kernel(x, kernel):
    global LAST_RESULTS
    x = np.asarray(x, dtype=np.float32)
    bdT, bd32T = make_mats(kernel)
    shards = x.reshape(NCORES, ROWS, W)
    nc = build_nc()
    in_maps = [
        {"x": shards[i], "bdT": bdT, "bd32T": bd32T} for i in range(NCORES)
    ]
    res = run_bass_kernel_spmd(
        nc, in_maps, core_ids=list(range(NCORES)), trace=TRACE
    )
    LAST_RESULTS = res
    out = np.stack([np.asarray(r["out"]) for r in res.results], axis=0)
    return out.reshape(B, C, H, W)


if __name__ == "__main__":
    rng = np.random.default_rng(0)
    x = rng.standard_normal((B, C, H, W)).astype(np.float32)
    import math
    i = np.arange(KSIZE)[:, None].astype(np.float64)
    j = np.arange(KSIZE)[None, :].astype(np.float64)
    scale = np.where(i == 0, math.sqrt(1.0 / KSIZE), math.sqrt(2.0 / KSIZE))
    km = (scale * np.cos((j + 0.5) * math.pi * i / KSIZE)).astype(np.float32)
    out = kernel(x, km)
    print(out.shape, out.dtype)

